# revision 1
# baseline (speedup 1.0000x reference)
"""DGM-net forward kernel for Trainium2, 8-core data parallel.

Network (per batch row x of width 101, n_nodes=512, 3 layers):
    S = tanh(x @ W0 + b0)
    for i in 0..2:
        Z = tanh(x @ Uz[i] + S @ Wz[i] + bz[i])
        G = tanh(x @ Ug[i] + S @ Wg[i] + bg[i])
        R = tanh(x @ Ur[i] + S @ Wr[i] + br[i])
        H = tanh(x @ Uh[i] + (S*R) @ Wh[i] + bh[i])
        S = (1-G)*H + Z*S
    out = S @ Wf + bf

Layout strategy: activations are kept feature-major (transposed:
[feature partitions, batch free]) so every matmul uses the weight matrix
in its NATURAL layout as the stationary lhsT operand and the activation
as the moving rhs: out^T[n,b] = sum_k W[k,n] S^T[k,b].  X is transposed
ONCE on the host (free, bit-exact) and shipped as the fp32r "XT" input,
so the device never transposes anything.

Batch is tiled into chunks of 512 (= one PSUM bank in fp32 and the fp32
moving-operand max).  All weights stay resident in SBUF; per chunk the
whole 3-layer network runs fused without touching DRAM except the X load
and the [1 x 512] output store.

Matmuls run as float32r (PE relaxed-precision fp32: 1 cycle/row vs plain
fp32's 4).  fp32r operands must be *produced* as fp32r, so weight DRAM
params are declared float32r (DMA passthrough) and activation producers
(tanh / DVE updates) write float32r directly.  Accumulation is fp32.
bf16 was tried and measured 1.54x SLOWER per matmul on HW (serial
weight-load per instruction that fp32r's self-loading mode overlaps), so
fp32r everywhere is the fastest correct configuration.
"""
import numpy as np
from contextlib import ExitStack

import concourse.bacc as bacc
import concourse.mybir as mybir
import concourse.tile as tile
from concourse.bass_utils import run_bass_kernel_spmd


N_CORES = 8
B_FULL = 65536
B = B_FULL // N_CORES      # rows per core
D = 101                    # input width
N = 512                    # n_nodes
L = 3                      # layers
BT = 512                   # batch chunk (free dim of matmuls)
NT = N // 128              # output-feature tiles per gate
KT = N // 128              # contraction tiles for S@W
FP = mybir.dt.float32
FR = mybir.dt.float32r

GATES = ("z", "g", "r", "h")


def _build(mm_dt=FR, weight_dt=FR, reps=1):
    nc = bacc.Bacc(None)
    Tanh = mybir.ActivationFunctionType.Tanh
    WDT = weight_dt                 # stationary (lhsT) weight dtype
    ADT = mm_dt                     # moving (rhs) activation dtype

    # X ships pre-transposed from the host ([feature, batch] feature-major,
    # fp32r passthrough): per-chunk xt tiles DMA straight into matmul
    # layout (2KB/partition contiguous), removing all 64 PE transposes
    # (16.4K cycles), the identity matrix, and 16 ACT copies.
    XTd = nc.declare_dram_parameter("XT", [D, B], ADT, isOutput=False)
    W0d = nc.declare_dram_parameter("W0", [D, N], WDT, isOutput=False)
    Ud = {g: nc.declare_dram_parameter(f"U{g}", [L, D, N], WDT, isOutput=False)
          for g in GATES}
    Wd = {g: nc.declare_dram_parameter(f"W{g}", [L, N, N], WDT, isOutput=False)
          for g in GATES}
    Wfd = nc.declare_dram_parameter("Wf", [N, 1], WDT, isOutput=False)
    # all biases pre-packed on host into one [128, 53] tensor (cols 0:4 =
    # b0 nt-major, cols 4+12g:+12 = per-gate (l,nt)-major, col 52 row 0 =
    # bf) so they cost ONE fast HWDGE transfer instead of six serial SWDGE
    # issues (~1us of gpsimd time each) ahead of the layer-0 weights.
    biascd = nc.declare_dram_parameter("biasc", [128, 53], FP, isOutput=False)
    OUT = nc.declare_dram_parameter("out", [B, 1], FP, isOutput=True)

    with tile.TileContext(nc) as tc, ExitStack() as ctx:
        consts = ctx.enter_context(tc.tile_pool(name="consts", bufs=1))
        xtpool = ctx.enter_context(tc.tile_pool(name="xt", bufs=3))
        spool = ctx.enter_context(tc.tile_pool(name="s", bufs=3))
        zpool = ctx.enter_context(tc.tile_pool(name="z", bufs=1))
        gpool = ctx.enter_context(tc.tile_pool(name="g", bufs=1))
        rpool = ctx.enter_context(tc.tile_pool(name="r", bufs=1))
        hpool = ctx.enter_context(tc.tile_pool(name="h", bufs=1))
        opool = ctx.enter_context(tc.tile_pool(name="o", bufs=2))
        psum = ctx.enter_context(tc.tile_pool(name="psum", bufs=7, space="PSUM"))
        psum_f = ctx.enter_context(tc.tile_pool(name="psum_f", bufs=1, space="PSUM"))

        # --- resident weights, all in natural (k-major) layout ---
        # Weight DMAs go through GpSimd's SWDGE queue: its sequencer has no
        # compute role, so weight streaming never blocks the ACT/SP
        # instruction streams (X loads / tanh).  Issued per layer in the
        # order the first chunk consumes them so the PE starts ASAP.
        def wdma(out, in_):
            nc.gpsimd.dma_start(out=out, in_=in_)

        # Layer-0 weight tiles stay per-gate so the first chunk starts as
        # soon as each gate's matrix lands; layers 1-2 are merged into one
        # tile (and ONE SWDGE dma) per gate — SWDGE issue time is ~0.7us
        # fixed + ~3ns/descriptor of serial gpsimd ucode per instruction,
        # so fewer/larger DMAs drain the stream several us sooner.
        w0 = consts.tile([D, N], WDT)
        biasct = consts.tile([128, 53], FP)
        u0, w0g, u12, w12 = {}, {}, {}, {}
        for g in GATES:
            u0[g] = consts.tile([D, N], WDT, name=f"u0_{g}")
            w0g[g] = consts.tile([128, KT, N], WDT, name=f"w0_{g}")
            u12[g] = consts.tile([D, L - 1, N], WDT, name=f"u12_{g}")
            w12[g] = consts.tile([128, L - 1, KT, N], WDT, name=f"w12_{g}")
        wf = consts.tile([128, KT], WDT)

        def u_ap(g, l, c0, c1):
            return u0[g][:, c0:c1] if l == 0 else u12[g][:, l - 1, c0:c1]

        def w_ap(g, l, kt, c0, c1):
            return (w0g[g][:, kt, c0:c1] if l == 0
                    else w12[g][:, l - 1, kt, c0:c1])

        def bias_ap(g, l, nt):
            i = 4 + 12 * GATES.index(g) + l * NT + nt
            return biasct[:, i:i + 1]

        def emit_weight_dmas():
            # biases: one small contiguous HWDGE transfer, first on the SP
            # queue so the chunk-0 tanh bias operand lands in ~0.3us
            nc.sync.dma_start(out=biasct[:], in_=biascd[:])
            wdma(w0[:], W0d[:])
            for g in GATES:
                wdma(u0[g][:], Ud[g][0].rearrange("p n -> p n"))
                # layer-0 W in halves: the kt 0-1 matmuls of each gate can
                # start while the kt 2-3 half is still in flight
                wdma(w0g[g][:, 0:2],
                     Wd[g][0, 0:256].rearrange("(kt p) n -> p kt n", p=128))
                wdma(w0g[g][:, 2:4],
                     Wd[g][0, 256:512].rearrange("(kt p) n -> p kt n", p=128))
            for g in GATES:
                wdma(u12[g][:], Ud[g][1:3].rearrange("l p n -> p l n"))
                wdma(w12[g][:, 0],
                     Wd[g][1].rearrange("(kt p) n -> p kt n", p=128))
            for g in GATES:
                wdma(w12[g][:, 1],
                     Wd[g][2].rearrange("(kt p) n -> p kt n", p=128))
            wdma(wf[:], Wfd[:].rearrange("(kt p) o -> p (kt o)", p=128))

        sub, mult = mybir.AluOpType.subtract, mybir.AluOpType.mult

        def f32(ap):            # read fp32r bits as plain fp32 (DVE/ACT reads)
            return ap.bitcast(FP) if ap.dtype == FR else ap

        def load_xt(c):
            # chunk 0 arrives in two halves so its batch-halved S0 starts
            # on the first; chunk 1 rides the ACT HWDGE queue so the two
            # startup chunks land in parallel; everything else stays on SP
            # (issue instructions for c>=3 block on xt buffer reuse, which
            # must not stall the ACT tanh stream).
            xt = xtpool.tile([D, BT], ADT)
            if c == 0:
                h = BT // 2
                nc.sync.dma_start(out=xt[:, 0:h], in_=XTd[:, 0:h])
                nc.sync.dma_start(out=xt[:, h:BT], in_=XTd[:, h:BT])
            else:
                eng = nc.scalar if c == 1 else nc.sync
                eng.dma_start(out=xt[:], in_=XTd[:, c * BT:(c + 1) * BT])
            return xt

        def emit_head_s0_part(xt, s, nts):
            # ---- S0 = tanh(X @ W0 + b0), nt-granular for interleaving ----
            for nt in nts:
                acc = psum.tile([128, BT], FP)
                nc.tensor.matmul(acc[:], w0[:, nt * 128:(nt + 1) * 128],
                                 xt[:], start=True, stop=True)
                nc.scalar.activation(s[:, nt, :], acc[:], Tanh,
                                     bias=biasct[:, nt:nt + 1])
            return s

        def emit_head_s0(xt):
            s = spool.tile([128, KT, BT], ADT, name="s")
            return emit_head_s0_part(xt, s, range(NT))

        def emit_layer(l, xt, s):
            if True:
                zt = zpool.tile([128, NT, BT], FP)
                gt = gpool.tile([128, NT, BT], FP)
                rt = rpool.tile([128, NT, BT], ADT)
                ht = hpool.tile([128, NT, BT], FP)
                # Z, G, R gates: tanh(X@U + S@W + b)
                for g, dest in (("z", zt), ("g", gt), ("r", rt)):
                    for nt in range(NT):
                        acc = psum.tile([128, BT], FP)
                        nc.tensor.matmul(
                            acc[:], u_ap(g, l, nt * 128, (nt + 1) * 128),
                            xt[:], start=True, stop=False)
                        for kt in range(KT):
                            nc.tensor.matmul(
                                acc[:],
                                w_ap(g, l, kt, nt * 128, (nt + 1) * 128),
                                s[:, kt, :], start=False, stop=(kt == KT - 1))
                        nc.scalar.activation(dest[:, nt, :], acc[:], Tanh,
                                             bias=bias_ap(g, l, nt))
                # R <- S*R (in place, rounded to fp32r; R only feeds (S*R)@Wh)
                for kt in range(KT):
                    nc.vector.tensor_mul(rt[:, kt, :], f32(s[:, kt, :]),
                                         f32(rt[:, kt, :]))
                # H = tanh(X@Uh + (S*R)@Wh + bh)
                for nt in range(NT):
                    acc = psum.tile([128, BT], FP)
                    nc.tensor.matmul(
                        acc[:], u_ap("h", l, nt * 128, (nt + 1) * 128),
                        xt[:], start=True, stop=False)
                    for kt in range(KT):
                        nc.tensor.matmul(
                            acc[:], w_ap("h", l, kt, nt * 128, (nt + 1) * 128),
                            rt[:, kt, :], start=False, stop=(kt == KT - 1))
                    nc.scalar.activation(ht[:, nt, :], acc[:], Tanh,
                                         bias=bias_ap("h", l, nt))
                # S = Z*S + (1-G)*H   (3 DVE ops per feature tile, in place)
                for nt in range(NT):
                    nc.vector.tensor_mul(zt[:, nt, :], zt[:, nt, :],
                                         f32(s[:, nt, :]))
                    nc.vector.scalar_tensor_tensor(
                        gt[:, nt, :], gt[:, nt, :], 1.0, ht[:, nt, :],
                        op0=sub, op1=mult)          # (G-1)*H
                    nc.vector.tensor_sub(s[:, nt, :], zt[:, nt, :], gt[:, nt, :])

        def emit_final_mms(s, kts, accf=None):
            # ---- out = S @ Wf: accumulate kt pieces into one PSUM row ----
            if accf is None:
                accf = psum_f.tile([1, BT], FP)
            for kt in kts:
                nc.tensor.matmul(accf[:], wf[:, kt:kt + 1], s[:, kt, :],
                                 start=(kt == 0), stop=(kt == KT - 1))
            return accf

        def emit_final_store(c, accf):
            r0 = c * BT
            ot = opool.tile([1, BT], FP)
            nc.vector.tensor_scalar_add(ot[:], accf[:], biasct[0:1, 52:53])
            nc.sync.dma_start(out=OUT[r0:r0 + BT, 0:1].rearrange("b o -> o b"),
                              in_=ot[:])

        def emit_final(c, s):
            emit_final_store(c, emit_final_mms(s, range(KT)))

        def emit_all():
            # Software-pipelined: chunk c+2's xt DMA issues at the top of
            # chunk c and its S0 matmuls run after chunk c's layer 1, so
            # the l1->l2 boundary has S-independent PE work covering the
            # ACT->DVE S-update chain.  Chunks 0 AND 1 are precomputed up
            # front: ~1.7us of weight-independent PE work covering the
            # layer-0 weight stream.
            n_chunks = B // BT
            xt = load_xt(0)
            nxt_xt = load_xt(1)
            # chunk-0 S0 batch-halved: each 256-col half still runs at
            # 1 cycle/row fp32r, so the PE starts ~1.2us earlier on the
            # first xt0 half instead of waiting for the full transfer
            s = spool.tile([128, KT, BT], ADT, name="s")
            for h in range(2):
                c0, c1 = h * 256, (h + 1) * 256
                for nt in range(NT):
                    acc = psum.tile([128, BT], FP, name="acc")
                    nc.tensor.matmul(acc[:, 0:256],
                                     w0[:, nt * 128:(nt + 1) * 128],
                                     xt[:, c0:c1], start=True, stop=True)
                    nc.scalar.activation(s[:, nt, c0:c1], acc[:, 0:256], Tanh,
                                         bias=biasct[:, nt:nt + 1])
            nxt_s = emit_head_s0(nxt_xt)
            pend = None          # (c, s, accf) of a deferred final
            for c in range(n_chunks):
                if c + 2 < n_chunks:
                    nxt2_xt = load_xt(c + 2)
                emit_layer(0, xt, s)
                # cover the l0->l1 boundary with half of chunk c+2's S0
                # (or, on the last chunk, the deferred final of chunk n-2:
                # the only S-independent PE work left)
                if c + 2 < n_chunks:
                    nxt2_s = spool.tile([128, KT, BT], ADT, name="s")
                    emit_head_s0_part(nxt2_xt, nxt2_s, range(0, 2))
                if pend is not None:
                    pend = (pend[0], pend[1],
                            emit_final_mms(pend[1], range(0, 2)))
                prev = (c, s)
                emit_layer(1, xt, s)
                if c + 2 < n_chunks:
                    emit_head_s0_part(nxt2_xt, nxt2_s, range(2, NT))
                elif pend is not None:
                    emit_final_mms(pend[1], range(2, KT), pend[2])
                    emit_final_store(pend[0], pend[2])
                    pend = None
                emit_layer(2, xt, s)
                if c == n_chunks - 2:
                    pend = prev
                else:
                    emit_final(*prev)
                xt, s = nxt_xt, nxt_s
                if c + 2 < n_chunks:
                    nxt_xt, nxt_s = nxt2_xt, nxt2_s

        emit_weight_dmas()
        if reps == 1:
            emit_all()
        else:           # device-side repetition loop, for benchmarking only
            with tc.For_i(0, reps):
                emit_all()

    nc.compile()
    return nc


_NC = None


def _get_nc():
    global _NC
    if _NC is None:
        _NC = _build()
    return _NC


WEIGHT_NAMES = ["W0"] + [f"U{g}" for g in GATES] + [f"W{g}" for g in GATES] + ["Wf"]
BIAS_NAMES = ["b0"] + [f"b{g}" for g in GATES] + ["bf"]


def prep_shared(inputs):
    shared = {n: np.ascontiguousarray(np.asarray(inputs[n], np.float32))
              for n in WEIGHT_NAMES}
    shared["identc"] = np.eye(128, dtype=np.float32)
    # pack every bias into one [128, 53] tensor (see biascd declaration)
    biasc = np.zeros((128, 53), np.float32)
    b0 = np.asarray(inputs["b0"], np.float32)
    biasc[:, 0:4] = b0.reshape(4, 128).T
    for gi, g in enumerate(GATES):
        bg = np.asarray(inputs[f"b{g}"], np.float32)      # [L, 1, N]
        biasc[:, 4 + 12 * gi: 16 + 12 * gi] = (
            bg.reshape(L, NT, 128).transpose(2, 0, 1).reshape(128, L * NT))
    biasc[0, 52] = np.asarray(inputs["bf"], np.float32)[0, 0]
    shared["biasc"] = biasc
    return shared


def prep_xt(Xcore):
    """[B, D] batch-major core shard -> [D, B] feature-major (bit-exact)."""
    return np.ascontiguousarray(np.asarray(Xcore, np.float32).T)


def _run(inputs, **kw):
    nc = _get_nc()
    shared = prep_shared(inputs)
    X = np.asarray(inputs["X"], np.float32)
    in_maps = [dict(shared, XT=prep_xt(X[i * B:(i + 1) * B]))
               for i in range(N_CORES)]
    res = run_bass_kernel_spmd(nc, in_maps, list(range(N_CORES)), **kw)
    out = np.concatenate([res.results[i]["out"] for i in range(N_CORES)], axis=0)
    return out, res


def kernel(**inputs) -> np.ndarray:
    out, _ = _run(inputs)
    return out



# revision 2
# speedup vs baseline: 1.0349x; 1.0349x over previous
"""DGM-net forward kernel for Trainium2, 8-core data parallel.

Network (per batch row x of width 101, n_nodes=512, 3 layers):
    S = tanh(x @ W0 + b0)
    for i in 0..2:
        Z = tanh(x @ Uz[i] + S @ Wz[i] + bz[i])
        G = tanh(x @ Ug[i] + S @ Wg[i] + bg[i])
        R = tanh(x @ Ur[i] + S @ Wr[i] + br[i])
        H = tanh(x @ Uh[i] + (S*R) @ Wh[i] + bh[i])
        S = (1-G)*H + Z*S
    out = S @ Wf + bf

Layout strategy: activations are kept feature-major (transposed:
[feature partitions, batch free]) so every matmul uses the weight matrix
in its NATURAL layout as the stationary lhsT operand and the activation
as the moving rhs: out^T[n,b] = sum_k W[k,n] S^T[k,b].  X is transposed
ONCE on the host (free, bit-exact) and shipped as the fp32r "XT" input
with an appended ones row (row 101 == 1.0), so every bias is FOLDED into
the matmul: the U/W0 stationaries carry the bias as row 101.  The device
never transposes anything and ACT instructions need no bias operand.

v2 (HW-calibrated): microbenchmarks on the real device measured
  fp32r matmul [128x128]x[128x512]  ~152 ns   (cost model: 213)
  ACT tanh [128,512] PSUM->SBUF     ~458 ns
  DVE tensor op [128,512]           ~321 ns
and ~235 ns of per-group semaphore/handoff overhead.  The kernel is PE
stream-bound in theory (603 us) but the v1 baseline measured 1138 us --
the gap is per-instruction/per-group sync overhead.  So v2 merges work
into wider instructions: PSUM accumulation groups cover TWO banks
([128, 2, 512]); one bias-free ACT reads both banks ([128, 1024]); DVE
elementwise ops run at half-gate width [128, 1024] or full-gate width.
This halves PE group count and cuts ACT instruction count 2x and DVE
instruction count 4x.

Matmuls run as float32r (PE relaxed-precision fp32: 1 cycle/row vs plain
fp32's 4).  fp32r operands must be *produced* as fp32r, so weight DRAM
params are declared float32r (DMA passthrough) and activation producers
(tanh / DVE updates) write float32r directly.  Accumulation is fp32.
"""
import numpy as np
from contextlib import ExitStack

import concourse.bacc as bacc
import concourse.mybir as mybir
import concourse.tile as tile
from concourse.bass_utils import run_bass_kernel_spmd


N_CORES = 8
B_FULL = 65536
B = B_FULL // N_CORES      # rows per core
D = 101                    # input width
DA = D + 1                 # augmented with ones row (bias fold)
N = 512                    # n_nodes
L = 3                      # layers
BT = 512                   # batch chunk (free dim of matmuls)
NT = N // 128              # output-feature tiles per gate
KT = N // 128              # contraction tiles for S@W
NP = NT // 2               # half-gate pairs
FP = mybir.dt.float32
FR = mybir.dt.float32r

GATES = ("z", "g", "r", "h")


def _build(reps=1):
    nc = bacc.Bacc(None)
    Tanh = mybir.ActivationFunctionType.Tanh

    # X ships pre-transposed from the host ([feature, batch] feature-major,
    # fp32r passthrough) with a trailing ones row; per-chunk xt tiles DMA
    # straight into matmul layout (2KB/partition contiguous).
    XTd = nc.declare_dram_parameter("XT", [DA, B], FR, isOutput=False)
    # U-type stationaries are host-augmented with the bias as row 101.
    W0d = nc.declare_dram_parameter("W0a", [DA, N], FR, isOutput=False)
    Ud = {g: nc.declare_dram_parameter(f"U{g}a", [L, DA, N], FR, isOutput=False)
          for g in GATES}
    Wd = {g: nc.declare_dram_parameter(f"W{g}", [L, N, N], FR, isOutput=False)
          for g in GATES}
    Wfd = nc.declare_dram_parameter("Wf", [N, 1], FR, isOutput=False)
    bfd = nc.declare_dram_parameter("bfc", [1, 1], FP, isOutput=False)
    OUT = nc.declare_dram_parameter("out", [B, 1], FP, isOutput=True)

    with tile.TileContext(nc) as tc, ExitStack() as ctx:
        consts = ctx.enter_context(tc.tile_pool(name="consts", bufs=1))
        xtpool = ctx.enter_context(tc.tile_pool(name="xt", bufs=3))
        spool = ctx.enter_context(tc.tile_pool(name="s", bufs=3))
        zpool = ctx.enter_context(tc.tile_pool(name="z", bufs=1))
        gpool = ctx.enter_context(tc.tile_pool(name="g", bufs=1))
        rpool = ctx.enter_context(tc.tile_pool(name="r", bufs=1))
        hpool = ctx.enter_context(tc.tile_pool(name="h", bufs=1))
        opool = ctx.enter_context(tc.tile_pool(name="o", bufs=2))
        # pair-granular PSUM: each tile spans TWO banks ([128, 2, 512] fp32)
        psum = ctx.enter_context(tc.tile_pool(name="psum", bufs=3, space="PSUM"))
        psum_f = ctx.enter_context(tc.tile_pool(name="psum_f", bufs=1, space="PSUM"))

        # --- resident weights, all in natural (k-major) layout ---
        # Weight DMAs go through GpSimd's SWDGE queue: its sequencer has no
        # compute role, so weight streaming never blocks the ACT/SP
        # instruction streams (X loads / tanh).  Issued per layer in the
        # order the first chunk consumes them so the PE starts ASAP.
        def wdma(out, in_):
            nc.gpsimd.dma_start(out=out, in_=in_)

        w0 = consts.tile([DA, N], FR)
        bfc = consts.tile([1, 1], FP)
        u0, w0g, u12, w12 = {}, {}, {}, {}
        for g in GATES:
            u0[g] = consts.tile([DA, N], FR, name=f"u0_{g}")
            w0g[g] = consts.tile([128, KT, N], FR, name=f"w0_{g}")
            u12[g] = consts.tile([DA, L - 1, N], FR, name=f"u12_{g}")
            w12[g] = consts.tile([128, L - 1, KT, N], FR, name=f"w12_{g}")
        wf = consts.tile([128, KT], FR)

        def u_ap(g, l, c0, c1):
            return u0[g][:, c0:c1] if l == 0 else u12[g][:, l - 1, c0:c1]

        def w_ap(g, l, kt, c0, c1):
            return (w0g[g][:, kt, c0:c1] if l == 0
                    else w12[g][:, l - 1, kt, c0:c1])

        def emit_weight_dmas():
            nc.sync.dma_start(out=bfc[:], in_=bfd[:])
            wdma(w0[:], W0d[:])
            for g in GATES:
                wdma(u0[g][:], Ud[g][0].rearrange("p n -> p n"))
                # layer-0 W in halves: the kt 0-1 matmuls of each gate can
                # start while the kt 2-3 half is still in flight
                wdma(w0g[g][:, 0:2],
                     Wd[g][0, 0:256].rearrange("(kt p) n -> p kt n", p=128))
                wdma(w0g[g][:, 2:4],
                     Wd[g][0, 256:512].rearrange("(kt p) n -> p kt n", p=128))
            for g in GATES:
                wdma(u12[g][:], Ud[g][1:3].rearrange("l p n -> p l n"))
                wdma(w12[g][:, 0],
                     Wd[g][1].rearrange("(kt p) n -> p kt n", p=128))
            for g in GATES:
                wdma(w12[g][:, 1],
                     Wd[g][2].rearrange("(kt p) n -> p kt n", p=128))
            wdma(wf[:], Wfd[:].rearrange("(kt p) o -> p (kt o)", p=128))

        sub, mult = mybir.AluOpType.subtract, mybir.AluOpType.mult

        def f32(ap):            # read fp32r bits as plain fp32 (DVE/ACT reads)
            return ap.bitcast(FP) if ap.dtype == FR else ap

        def load_xt(c):
            # chunk 0 arrives in two halves so its batch-halved S0 starts
            # on the first; chunk 1 rides the ACT HWDGE queue so the two
            # startup chunks land in parallel; everything else stays on SP
            # (issue instructions for c>=3 block on xt buffer reuse, which
            # must not stall the ACT tanh stream).
            xt = xtpool.tile([DA, BT], FR)
            if c == 0:
                h = BT // 2
                nc.sync.dma_start(out=xt[:, 0:h], in_=XTd[:, 0:h])
                nc.sync.dma_start(out=xt[:, h:BT], in_=XTd[:, h:BT])
            else:
                eng = nc.scalar if c == 1 else nc.sync
                eng.dma_start(out=xt[:], in_=XTd[:, c * BT:(c + 1) * BT])
            return xt

        def emit_head_s0_part(xt, s, nps):
            # ---- S0 = tanh(X_aug @ W0_aug), pair-granular ----
            for np_ in nps:
                acc = psum.tile([128, 2, BT], FP)
                for i in range(2):
                    nt = 2 * np_ + i
                    nc.tensor.matmul(acc[:, i, :],
                                     w0[:, nt * 128:(nt + 1) * 128],
                                     xt[:], start=True, stop=True)
                nc.scalar.activation(s[:, 2 * np_:2 * np_ + 2, :], acc[:], Tanh)
            return s

        def emit_head_s0(xt):
            s = spool.tile([128, KT, BT], FR, name="s")
            return emit_head_s0_part(xt, s, range(NP))

        def emit_gate_pair(g, l, xt, src, np_, dest):
            # one PSUM pair-group: out-features [2*np_*128, (2*np_+2)*128),
            # contraction X_aug (bias folded) + 4 S-kt tiles; one ACT.
            acc = psum.tile([128, 2, BT], FP)
            for i in range(2):
                nt = 2 * np_ + i
                nc.tensor.matmul(
                    acc[:, i, :], u_ap(g, l, nt * 128, (nt + 1) * 128),
                    xt[:], start=True, stop=False)
                for kt in range(KT):
                    nc.tensor.matmul(
                        acc[:, i, :],
                        w_ap(g, l, kt, nt * 128, (nt + 1) * 128),
                        src[:, kt, :], start=False, stop=(kt == KT - 1))
            nc.scalar.activation(dest[:, 2 * np_:2 * np_ + 2, :], acc[:], Tanh)

        def emit_layer(l, xt, s):
            zt = zpool.tile([128, NT, BT], FP)
            gt = gpool.tile([128, NT, BT], FP)
            rt = rpool.tile([128, NT, BT], FR)
            ht = hpool.tile([128, NT, BT], FP)
            # Z, G, R gates: tanh(X@U + S@W)  [bias folded into U]
            for g, dest in (("z", zt), ("g", gt), ("r", rt)):
                for np_ in range(NP):
                    emit_gate_pair(g, l, xt, s, np_, dest)
            # R <- S*R at half-gate width (in place, rounded to fp32r;
            # R only feeds (S*R)@Wh)
            for hf in range(2):
                nc.vector.tensor_mul(rt[:, 2 * hf:2 * hf + 2, :],
                                     f32(s[:, 2 * hf:2 * hf + 2, :]),
                                     f32(rt[:, 2 * hf:2 * hf + 2, :]))
            # H = tanh(X@Uh + (S*R)@Wh)
            for np_ in range(NP):
                emit_gate_pair("h", l, xt, rt, np_, ht)
            # S = Z*S + (1-G)*H  as 3 DVE ops per half-gate, in place
            for hf in range(2):
                c = slice(2 * hf, 2 * hf + 2)
                nc.vector.tensor_mul(zt[:, c, :], zt[:, c, :], f32(s[:, c, :]))
                nc.vector.scalar_tensor_tensor(
                    gt[:, c, :], gt[:, c, :], 1.0, ht[:, c, :],
                    op0=sub, op1=mult)          # (G-1)*H
                nc.vector.tensor_sub(s[:, c, :], zt[:, c, :], gt[:, c, :])

        def emit_final_mms(s, kts, accf=None):
            # ---- out = S @ Wf: accumulate kt pieces into one PSUM row ----
            if accf is None:
                accf = psum_f.tile([1, BT], FP)
            for kt in kts:
                nc.tensor.matmul(accf[:], wf[:, kt:kt + 1], s[:, kt, :],
                                 start=(kt == 0), stop=(kt == KT - 1))
            return accf

        def emit_final_store(c, accf):
            r0 = c * BT
            ot = opool.tile([1, BT], FP)
            nc.vector.tensor_scalar_add(ot[:], accf[:], bfc[0:1, 0:1])
            nc.sync.dma_start(out=OUT[r0:r0 + BT, 0:1].rearrange("b o -> o b"),
                              in_=ot[:])

        def emit_final(c, s):
            emit_final_store(c, emit_final_mms(s, range(KT)))

        def emit_all():
            # Software-pipelined: chunk c+2's xt DMA issues at the top of
            # chunk c and its S0 matmuls run after chunk c's layer 1, so
            # the l1->l2 boundary has S-independent PE work covering the
            # ACT->DVE S-update chain.  Chunks 0 AND 1 are precomputed up
            # front: ~1.7us of weight-independent PE work covering the
            # layer-0 weight stream.
            n_chunks = B // BT
            xt = load_xt(0)
            nxt_xt = load_xt(1)
            # chunk-0 S0 batch-halved: each 256-col half still runs at
            # 1 cycle/row fp32r, so the PE starts ~1.2us earlier on the
            # first xt0 half instead of waiting for the full transfer
            s = spool.tile([128, KT, BT], FR, name="s")
            for h in range(2):
                c0, c1 = h * 256, (h + 1) * 256
                for np_ in range(NP):
                    acc = psum.tile([128, 2, BT], FP, name="acc")
                    for i in range(2):
                        nt = 2 * np_ + i
                        nc.tensor.matmul(acc[:, i, 0:256],
                                         w0[:, nt * 128:(nt + 1) * 128],
                                         xt[:, c0:c1], start=True, stop=True)
                    nc.scalar.activation(
                        s[:, 2 * np_:2 * np_ + 2, c0:c1],
                        acc[:, :, 0:256], Tanh)
            nxt_s = emit_head_s0(nxt_xt)
            pend = None          # (c, s, accf) of a deferred final
            for c in range(n_chunks):
                if c + 2 < n_chunks:
                    nxt2_xt = load_xt(c + 2)
                emit_layer(0, xt, s)
                # cover the l0->l1 boundary with half of chunk c+2's S0
                # (or, on the last chunk, the deferred final of chunk n-2:
                # the only S-independent PE work left)
                if c + 2 < n_chunks:
                    nxt2_s = spool.tile([128, KT, BT], FR, name="s")
                    emit_head_s0_part(nxt2_xt, nxt2_s, range(0, 1))
                if pend is not None:
                    pend = (pend[0], pend[1],
                            emit_final_mms(pend[1], range(0, 2)))
                prev = (c, s)
                emit_layer(1, xt, s)
                if c + 2 < n_chunks:
                    emit_head_s0_part(nxt2_xt, nxt2_s, range(1, NP))
                elif pend is not None:
                    emit_final_mms(pend[1], range(2, KT), pend[2])
                    emit_final_store(pend[0], pend[2])
                    pend = None
                emit_layer(2, xt, s)
                if c == n_chunks - 2:
                    pend = prev
                else:
                    emit_final(*prev)
                xt, s = nxt_xt, nxt_s
                if c + 2 < n_chunks:
                    nxt_xt, nxt_s = nxt2_xt, nxt2_s

        emit_weight_dmas()
        if reps == 1:
            emit_all()
        else:           # device-side repetition loop, for benchmarking only
            with tc.For_i(0, reps):
                emit_all()

    nc.compile()
    return nc


_NC = None


def _get_nc():
    global _NC
    if _NC is None:
        _NC = _build()
    return _NC


def prep_shared(inputs):
    """Augment U-type weights with their bias row; pass W/Wf through."""
    shared = {}
    for g in GATES:
        shared[f"W{g}"] = np.ascontiguousarray(
            np.asarray(inputs[f"W{g}"], np.float32))
        U = np.asarray(inputs[f"U{g}"], np.float32)          # [L, D, N]
        b = np.asarray(inputs[f"b{g}"], np.float32)          # [L, 1, N]
        shared[f"U{g}a"] = np.ascontiguousarray(
            np.concatenate([U, b.reshape(L, 1, N)], axis=1))  # [L, DA, N]
    W0 = np.asarray(inputs["W0"], np.float32)                # [D, N]
    b0 = np.asarray(inputs["b0"], np.float32)                # [1, N]
    shared["W0a"] = np.ascontiguousarray(np.concatenate([W0, b0], axis=0))
    shared["Wf"] = np.ascontiguousarray(np.asarray(inputs["Wf"], np.float32))
    shared["bfc"] = np.asarray(inputs["bf"], np.float32).reshape(1, 1)
    return shared


def prep_xt(Xcore):
    """[B, D] batch-major core shard -> [DA, B] feature-major with ones row."""
    Xt = np.asarray(Xcore, np.float32).T                     # [D, B]
    ones = np.ones((1, Xt.shape[1]), np.float32)
    return np.ascontiguousarray(np.concatenate([Xt, ones], axis=0))


def _run(inputs, **kw):
    nc = _get_nc()
    shared = prep_shared(inputs)
    X = np.asarray(inputs["X"], np.float32)
    in_maps = [dict(shared, XT=prep_xt(X[i * B:(i + 1) * B]))
               for i in range(N_CORES)]
    res = run_bass_kernel_spmd(nc, in_maps, list(range(N_CORES)), **kw)
    out = np.concatenate([res.results[i]["out"] for i in range(N_CORES)], axis=0)
    return out, res


def kernel(**inputs) -> np.ndarray:
    out, _ = _run(inputs)
    return out
